# revision 11
# baseline (speedup 1.0000x reference)
"""Trainium2 Bass kernel for nn_Encoder (dense MLP 6->8->4->2->1 + softplus).

Strategy: pure data parallel over 8 NeuronCores. The host pre-arranges x into
a "block" layout in which each group of 16 consecutive rows becomes one
streamed PE column (features x rows on SBUF partitions), so the whole MLP runs
as 4 chained block-diagonal matmuls on the TensorEngine with no on-chip
transposes. ReLU + bias are fused into the PSUM->SBUF evacuation copies
(split between VectorE and ScalarE); softplus = Ln(Exp(z)+1) on ScalarE.
Activations and x travel as bf16 (fp32 PSUM accumulation); output is fp32.
"""

import os
import sys

sys.path.insert(0, "/opt/trn_rl_repo")

import numpy as np

import concourse.bass as bass
import concourse.mybir as mybir
import concourse.tile as tile
from concourse.bass_utils import run_bass_kernel_spmd

# ---------------------------------------------------------------- geometry
N_CORES = 8
N_ROWS = 4194304
ROWS_PER_CORE = N_ROWS // N_CORES          # 524288
G = 16                                      # rows per PE column
COLS_PER_CORE = ROWS_PER_CORE // G          # 32768 columns of 16 rows
ST_COLS = 2048                              # columns per supertile
N_ST = COLS_PER_CORE // ST_COLS             # 16 supertiles per core
FD = 512                                    # matmul free dim / PSUM bank
BF16 = mybir.dt.bfloat16
F32 = mybir.dt.float32

# walrus in this container rejects instructions carrying more than
# _MAX_WAITS sync waits; split the surplus onto same-engine NoOps placed
# immediately before the instruction.
_MAX_WAITS = 1


def _split_multi_waits(nc, max_waits=_MAX_WAITS):
    ctr = 0
    for f in nc.m.functions:
        for bb in f.blocks:
            out = []
            for inst in bb.instructions:
                si = getattr(inst, "sync_info", None)
                if si is not None and si.on_wait and len(si.on_wait) > max_waits:
                    waits = list(si.on_wait)
                    split = len(waits) - max_waits
                    for i in range(0, split, max_waits):
                        nop = mybir.InstNoOp(
                            name=f"waitsplit-{ctr}", ins=[], outs=[]
                        )
                        ctr += 1
                        nop.engine = inst.engine
                        nop.sync_info = mybir.SyncInfo(
                            on_wait=waits[i : min(i + max_waits, split)],
                            on_update=[],
                        )
                        out.append(nop)
                    inst.sync_info = mybir.SyncInfo(
                        on_wait=waits[split:], on_update=list(si.on_update)
                    )
                out.append(inst)
            bb.instructions[:] = out


# Set KERNEL_TRACE=1 to neuron-profile the run; kernel() then stashes the
# BassKernelResults (exec_time_ns, trace paths) in LAST_RESULTS.
TRACE = os.environ.get("KERNEL_TRACE", "0") == "1"
LAST_RESULTS = None


def _register_ntff_hook():
    """The image's antenv lacks axon_hooks; inject it and register the ctypes
    NTFF profile hook so run_bass_kernel_spmd(trace=True) works under axon."""
    import types

    if "antenv.axon_hooks" not in sys.modules:
        mod = types.ModuleType("antenv.axon_hooks")
        mod._hook = None

        def set_axon_ntff_profile_hook(h, _mod=mod):
            _mod._hook = h

        def get_axon_ntff_profile_hook(_mod=mod):
            return _mod._hook

        mod.set_axon_ntff_profile_hook = set_axon_ntff_profile_hook
        mod.get_axon_ntff_profile_hook = get_axon_ntff_profile_hook
        sys.modules["antenv.axon_hooks"] = mod
        import antenv

        antenv.axon_hooks = mod
    mod = sys.modules["antenv.axon_hooks"]
    if mod.get_axon_ntff_profile_hook() is None:
        try:
            from trn_agent_boot.trn_boot import _ntff_profile_via_ctypes

            mod.set_axon_ntff_profile_hook(
                _ntff_profile_via_ctypes("/opt/axon/libaxon_pjrt.so")
            )
        except Exception:
            pass


# ---------------------------------------------------------------- program
def build_program(n_st=N_ST, st_cols=ST_COLS, b9=0.0, split_waits=True):
    """One SPMD NeuronCore program; all 8 cores run it on their own shard."""
    nc = bass.Bass("TRN2", target_bir_lowering=False, debug=False,
                   num_devices=N_CORES)

    xb = nc.dram_tensor("xb", [n_st, 96, st_cols], BF16,
                        kind="ExternalInput").ap()
    w1 = nc.dram_tensor("w1blk", [96, 128], BF16, kind="ExternalInput").ap()
    w2 = nc.dram_tensor("w2blk", [128, 64], BF16, kind="ExternalInput").ap()
    w3 = nc.dram_tensor("w3blk", [128, 64], BF16, kind="ExternalInput").ap()
    w4 = nc.dram_tensor("w4blk", [128, 64], BF16, kind="ExternalInput").ap()
    bv = nc.dram_tensor("bvecs", [128, 4], F32, kind="ExternalInput").ap()
    out = nc.dram_tensor("out", [n_st // 2, 128, FD], F32,
                         kind="ExternalOutput").ap()

    FD2 = 2 * FD
    Relu = mybir.ActivationFunctionType.Relu
    Exp = mybir.ActivationFunctionType.Exp
    Ln = mybir.ActivationFunctionType.Ln
    ADD = mybir.AluOpType.add
    MAX = mybir.AluOpType.max

    with tile.TileContext(nc) as tc:
        with (
            tc.tile_pool(name="consts", bufs=1) as cpool,
            tc.tile_pool(name="xin", bufs=3) as xpool,
            tc.tile_pool(name="z1r", bufs=4) as z1pool,
            tc.tile_pool(name="z2r", bufs=2) as z2pool,
            tc.tile_pool(name="z3r", bufs=2) as z3pool,
            tc.tile_pool(name="zexp", bufs=2) as zepool,
            tc.tile_pool(name="zout", bufs=2) as zopool,
            tc.tile_pool(name="psbig", bufs=2, space="PSUM") as psbig,
            tc.tile_pool(name="ps3", bufs=2, space="PSUM") as ps3,
            tc.tile_pool(name="ps4", bufs=1, space="PSUM") as ps4,
        ):
            # --- constants into SBUF once
            w1t = cpool.tile([96, 128], BF16, tag="w1")
            nc.sync.dma_start(w1t[:], w1[:])
            w2t = cpool.tile([128, 64], BF16, tag="w2")
            nc.sync.dma_start(w2t[:], w2[:])
            w3t = cpool.tile([128, 64], BF16, tag="w3")
            nc.sync.dma_start(w3t[:], w3[:])
            w4t = cpool.tile([128, 64], BF16, tag="w4")
            nc.sync.dma_start(w4t[:], w4[:])
            bvt = cpool.tile([128, 4], F32, tag="bv")
            nc.sync.dma_start(bvt[:], bv[:])
            b1v, b2v, b3v, b9v = (bvt[:, 0:1], bvt[:, 1:2], bvt[:, 2:3],
                                  bvt[:, 3:4])

            def relu_copy(dst, src, bias_ap, on_act):
                if on_act:
                    nc.scalar.activation(dst, src, Relu, bias=bias_ap,
                                         scale=1.0)
                else:
                    nc.vector.tensor_scalar(dst, src, bias_ap, 0.0, ADD, MAX)

            z4p = None
            for s in range(n_st):
                xt = xpool.tile([96, st_cols], BF16, tag="x")
                nc.sync.dma_start(xt[:, :], xb[s])

                # layer 1: two [96,128]^T @ [96,512] per 2-bank PSUM tile
                z1rs = []
                for half in range(2):           # z1a (cols 0:1024), z1b
                    z1p = psbig.tile([128, FD2], F32, tag="zz")
                    for i in range(2):
                        c0 = half * FD2 + i * FD
                        nc.tensor.matmul(
                            z1p[:, i * FD : (i + 1) * FD],
                            w1t[:],
                            xt[:, c0 : c0 + FD],
                            start=True, stop=True,
                        )
                    z1r = z1pool.tile([128, FD2], BF16, tag="z1r")
                    # DVE takes z1a, ACT takes z1b
                    relu_copy(z1r[:], z1p[:], b1v, on_act=(half == 1))
                    z1rs.append(z1r)

                # layer 2: 4 matmuls fill one 2-bank PSUM tile
                z2p = psbig.tile([128, FD2], F32, tag="zz")
                for half in range(2):
                    for i in range(2):
                        nc.tensor.matmul(
                            z2p[i * 64 : i * 64 + 64,
                                half * FD : (half + 1) * FD],
                            w2t[:],
                            z1rs[half][:, i * FD : (i + 1) * FD],
                            start=True, stop=True,
                        )
                z2r = z2pool.tile([128, FD2], BF16, tag="z2r")
                relu_copy(z2r[:], z2p[:], b2v, on_act=False)   # DVE

                # layer 3
                z3p = ps3.tile([128, FD], F32, tag="z3")
                for u in range(2):
                    nc.tensor.matmul(
                        z3p[u * 64 : u * 64 + 64, :],
                        w3t[:],
                        z2r[:, u * FD : (u + 1) * FD],
                        start=True, stop=True,
                    )
                z3r = z3pool.tile([128, FD], BF16, tag="z3r")
                relu_copy(z3r[:], z3p[:], b3v, on_act=True)    # ACT

                # layer 4: supertile pair shares one PSUM tile
                if s % 2 == 0:
                    z4p = ps4.tile([128, FD], F32, tag="z4")
                half4 = (s % 2) * 64
                nc.tensor.matmul(z4p[half4 : half4 + 64, :], w4t[:], z3r[:],
                                 start=True, stop=True)

                if s % 2 == 1:
                    ze = zepool.tile([128, FD], F32, tag="ze")
                    nc.scalar.activation(ze[:], z4p[:], Exp, bias=b9v,
                                         scale=1.0)
                    zo = zopool.tile([128, FD], F32, tag="zo")
                    nc.scalar.activation(zo[:], ze[:], Ln, bias=1.0, scale=1.0)
                    nc.sync.dma_start(out[s // 2], zo[:])

    if split_waits:
        _split_multi_waits(nc)
    return nc


# ---------------------------------------------------------------- host side
def _block_weights(W1, W7, W8, W9):
    w1blk = np.zeros((96, 128), np.float32)
    for r in range(16):
        w1blk[r * 6 : r * 6 + 6, r * 8 : r * 8 + 8] = W1.T
    w2blk = np.zeros((128, 64), np.float32)
    for r in range(16):
        w2blk[r * 8 : r * 8 + 8, r * 4 : r * 4 + 4] = W7.T
    w3blk = np.zeros((128, 64), np.float32)
    for h in range(2):
        for r in range(16):
            w3blk[h * 64 + r * 4 : h * 64 + r * 4 + 4,
                  h * 32 + r * 2 : h * 32 + r * 2 + 2] = W8.T
    w4blk = np.zeros((128, 64), np.float32)
    for q in range(2):
        for h in range(2):
            for r in range(16):
                w4blk[q * 64 + h * 32 + r * 2 : q * 64 + h * 32 + r * 2 + 2,
                      q * 32 + h * 16 + r] = W9.T[:, 0]
    return w1blk, w2blk, w3blk, w4blk


def kernel(x, W1, b1, W7, b7, W8, b8, W9, b9):
    import ml_dtypes

    x = np.ascontiguousarray(np.asarray(x, dtype=np.float32))
    W1, b1 = np.asarray(W1, np.float32), np.asarray(b1, np.float32)
    W7, b7 = np.asarray(W7, np.float32), np.asarray(b7, np.float32)
    W8, b8 = np.asarray(W8, np.float32), np.asarray(b8, np.float32)
    W9, b9 = np.asarray(W9, np.float32), np.asarray(b9, np.float32)

    bf = ml_dtypes.bfloat16
    w1blk, w2blk, w3blk, w4blk = _block_weights(W1, W7, W8, W9)
    w1blk, w2blk = w1blk.astype(bf), w2blk.astype(bf)
    w3blk, w4blk = w3blk.astype(bf), w4blk.astype(bf)
    bvecs = np.stack(
        [
            b1[np.arange(128) % 8],
            b7[np.arange(128) % 4],
            b8[np.arange(128) % 2],
            np.full(128, float(b9[0])),
        ],
        axis=1,
    ).astype(np.float32)

    # [N,6] -> per core [n_st, 96, st_cols]: block layout, partition = r*6+k
    xb = (
        x.reshape(N_CORES, N_ST, ST_COLS, G, 6)
        .transpose(0, 1, 3, 4, 2)
        .reshape(N_CORES, N_ST, 96, ST_COLS)
        .astype(bf)
    )
    xb = np.ascontiguousarray(xb)

    nc = build_program(b9=float(b9[0]))
    in_maps = [
        {
            "xb": xb[c],
            "w1blk": w1blk,
            "w2blk": w2blk,
            "w3blk": w3blk,
            "w4blk": w4blk,
            "bvecs": bvecs,
        }
        for c in range(N_CORES)
    ]
    kwargs = {}
    if TRACE:
        _register_ntff_hook()
        kwargs["trace"] = True
    res = run_bass_kernel_spmd(nc, in_maps, list(range(N_CORES)), **kwargs)
    global LAST_RESULTS
    LAST_RESULTS = res

    # out[c] is [8, 128, 512]; row = ((((P*2+so)*2+q)*2+h)*512+c)*16+r
    outs = []
    for c in range(N_CORES):
        arr = res.results[c]["out"]
        arr = (
            arr.reshape(N_ST // 2, 2, 2, 2, G, FD)
            .transpose(0, 1, 2, 3, 5, 4)
            .reshape(ROWS_PER_CORE, 1)
        )
        outs.append(arr)
    return np.ascontiguousarray(np.concatenate(outs, axis=0))


# revision 13
# speedup vs baseline: 1.6042x; 1.6042x over previous
"""Trainium2 Bass kernel for nn_Encoder (dense MLP 6->8->4->2->1 + softplus).

Strategy: pure data parallel over 8 NeuronCores. The host pre-arranges x into
a "block" layout in which each group of 16 consecutive rows becomes one
streamed PE column (features x rows on SBUF partitions), so the whole MLP runs
as 4 chained block-diagonal matmuls on the TensorEngine with no on-chip
transposes. ReLU + bias are fused into the PSUM->SBUF evacuation copies
(split between VectorE and ScalarE); softplus = Ln(Exp(z)+1) on ScalarE.
Activations and x travel as bf16 (fp32 PSUM accumulation); output is fp32.
"""

import os
import sys

sys.path.insert(0, "/opt/trn_rl_repo")

import numpy as np

import concourse.bass as bass
import concourse.mybir as mybir
import concourse.tile as tile
from concourse.bass_utils import run_bass_kernel_spmd

# ---------------------------------------------------------------- geometry
N_CORES = 8
N_ROWS = 4194304
ROWS_PER_CORE = N_ROWS // N_CORES          # 524288
G = 16                                      # rows per PE column
COLS_PER_CORE = ROWS_PER_CORE // G          # 32768 columns of 16 rows
ST_COLS = 2048                              # columns per supertile
N_ST = COLS_PER_CORE // ST_COLS             # 16 supertiles per core
FD = 512                                    # matmul free dim / PSUM bank
BF16 = mybir.dt.bfloat16
F32 = mybir.dt.float32

# walrus in this container rejects instructions carrying more than
# _MAX_WAITS sync waits; split the surplus onto same-engine NoOps placed
# immediately before the instruction.
_MAX_WAITS = 1


def _split_multi_waits(nc, max_waits=_MAX_WAITS):
    ctr = 0
    for f in nc.m.functions:
        for bb in f.blocks:
            out = []
            for inst in bb.instructions:
                si = getattr(inst, "sync_info", None)
                if si is not None and si.on_wait and len(si.on_wait) > max_waits:
                    waits = list(si.on_wait)
                    split = len(waits) - max_waits
                    for i in range(0, split, max_waits):
                        nop = mybir.InstNoOp(
                            name=f"waitsplit-{ctr}", ins=[], outs=[]
                        )
                        ctr += 1
                        nop.engine = inst.engine
                        nop.sync_info = mybir.SyncInfo(
                            on_wait=waits[i : min(i + max_waits, split)],
                            on_update=[],
                        )
                        out.append(nop)
                    inst.sync_info = mybir.SyncInfo(
                        on_wait=waits[split:], on_update=list(si.on_update)
                    )
                out.append(inst)
            bb.instructions[:] = out


# Set KERNEL_TRACE=1 to neuron-profile the run; kernel() then stashes the
# BassKernelResults (exec_time_ns, trace paths) in LAST_RESULTS.
TRACE = os.environ.get("KERNEL_TRACE", "0") == "1"
LAST_RESULTS = None


def _register_ntff_hook():
    """The image's antenv lacks axon_hooks; inject it and register the ctypes
    NTFF profile hook so run_bass_kernel_spmd(trace=True) works under axon."""
    import types

    if "antenv.axon_hooks" not in sys.modules:
        mod = types.ModuleType("antenv.axon_hooks")
        mod._hook = None

        def set_axon_ntff_profile_hook(h, _mod=mod):
            _mod._hook = h

        def get_axon_ntff_profile_hook(_mod=mod):
            return _mod._hook

        mod.set_axon_ntff_profile_hook = set_axon_ntff_profile_hook
        mod.get_axon_ntff_profile_hook = get_axon_ntff_profile_hook
        sys.modules["antenv.axon_hooks"] = mod
        import antenv

        antenv.axon_hooks = mod
    mod = sys.modules["antenv.axon_hooks"]
    if mod.get_axon_ntff_profile_hook() is None:
        try:
            from trn_agent_boot.trn_boot import _ntff_profile_via_ctypes

            mod.set_axon_ntff_profile_hook(
                _ntff_profile_via_ctypes("/opt/axon/libaxon_pjrt.so")
            )
        except Exception:
            pass


# ---------------------------------------------------------------- program
def build_program(n_st=N_ST, st_cols=ST_COLS, b9=0.0, split_waits=True):
    """One SPMD NeuronCore program; all 8 cores run it on their own shard."""
    nc = bass.Bass("TRN2", target_bir_lowering=False, debug=False,
                   num_devices=N_CORES)

    xb = nc.dram_tensor("xb", [n_st, 96, st_cols], BF16,
                        kind="ExternalInput").ap()
    w1 = nc.dram_tensor("w1blk", [96, 128], BF16, kind="ExternalInput").ap()
    w2 = nc.dram_tensor("w2blk", [128, 64], BF16, kind="ExternalInput").ap()
    w3 = nc.dram_tensor("w3blk", [128, 64], BF16, kind="ExternalInput").ap()
    w4 = nc.dram_tensor("w4blk", [128, 64], BF16, kind="ExternalInput").ap()
    bv = nc.dram_tensor("bvecs", [128, 4], F32, kind="ExternalInput").ap()
    out = nc.dram_tensor("out", [n_st // 2, 128, FD], F32,
                         kind="ExternalOutput").ap()

    FD2 = 2 * FD
    Relu = mybir.ActivationFunctionType.Relu
    Exp = mybir.ActivationFunctionType.Exp
    Ln = mybir.ActivationFunctionType.Ln
    ADD = mybir.AluOpType.add
    MAX = mybir.AluOpType.max

    with tile.TileContext(nc) as tc:
        with (
            tc.tile_pool(name="consts", bufs=1) as cpool,
            tc.tile_pool(name="xin", bufs=3) as xpool,
            tc.tile_pool(name="z1r", bufs=4) as z1pool,
            tc.tile_pool(name="z2r", bufs=3) as z2pool,
            tc.tile_pool(name="z3r", bufs=3) as z3pool,
            tc.tile_pool(name="zexp", bufs=2) as zepool,
            tc.tile_pool(name="zout", bufs=2) as zopool,
            tc.tile_pool(name="psbig", bufs=3, space="PSUM") as psbig,
            tc.tile_pool(name="ps3", bufs=1, space="PSUM") as ps3,
            tc.tile_pool(name="ps4", bufs=1, space="PSUM") as ps4,
        ):
            # --- constants into SBUF once
            w1t = cpool.tile([96, 128], BF16, tag="w1")
            nc.sync.dma_start(w1t[:], w1[:])
            w2t = cpool.tile([128, 64], BF16, tag="w2")
            nc.sync.dma_start(w2t[:], w2[:])
            w3t = cpool.tile([128, 64], BF16, tag="w3")
            nc.sync.dma_start(w3t[:], w3[:])
            w4t = cpool.tile([128, 64], BF16, tag="w4")
            nc.sync.dma_start(w4t[:], w4[:])
            bvt = cpool.tile([128, 4], F32, tag="bv")
            nc.sync.dma_start(bvt[:], bv[:])
            b1v, b2v, b3v, b9v = (bvt[:, 0:1], bvt[:, 1:2], bvt[:, 2:3],
                                  bvt[:, 3:4])

            def relu_copy(dst, src, bias_ap, on_act):
                if on_act:
                    nc.scalar.activation(dst, src, Relu, bias=bias_ap,
                                         scale=1.0)
                else:
                    nc.vector.tensor_scalar(dst, src, bias_ap, 0.0, ADD, MAX)

            # Software-pipelined across supertiles: in iteration i we emit
            # layer-1 for supertile i, layer-2 for i-1, layer-3 for i-2 and
            # layer-4 for i-3, so every matmul's inputs were produced a full
            # iteration earlier and the PE never stalls on a fresh copy.
            z1rs = {}
            z2rs = {}
            z3rs = {}
            z4p = None
            for i in range(n_st + 3):
                if i < n_st:
                    s = i
                    xt = xpool.tile([96, st_cols], BF16, tag="x")
                    nc.sync.dma_start(xt[:, :], xb[s])
                    # layer 1: two [96,128]^T @ [96,512] per 2-bank PSUM tile
                    z1rs[s] = []
                    for half in range(2):       # z1a (cols 0:1024), z1b
                        z1p = psbig.tile([128, FD2], F32, tag="zz")
                        for j in range(2):
                            c0 = half * FD2 + j * FD
                            nc.tensor.matmul(
                                z1p[:, j * FD : (j + 1) * FD],
                                w1t[:],
                                xt[:, c0 : c0 + FD],
                                start=True, stop=True,
                            )
                        z1r = z1pool.tile([128, FD2], BF16, tag="z1r")
                        relu_copy(z1r[:], z1p[:], b1v, on_act=(half == 1))
                        z1rs[s].append(z1r)

                if i >= 1 and i - 1 < n_st:
                    s = i - 1
                    # layer 2: 4 matmuls fill one 2-bank PSUM tile
                    z2p = psbig.tile([128, FD2], F32, tag="zz")
                    for half in range(2):
                        for j in range(2):
                            nc.tensor.matmul(
                                z2p[j * 64 : j * 64 + 64,
                                    half * FD : (half + 1) * FD],
                                w2t[:],
                                z1rs[s][half][:, j * FD : (j + 1) * FD],
                                start=True, stop=True,
                            )
                    del z1rs[s]
                    z2r = z2pool.tile([128, FD2], BF16, tag="z2r")
                    relu_copy(z2r[:], z2p[:], b2v, on_act=False)   # DVE
                    z2rs[s] = z2r

                if i >= 2 and i - 2 < n_st:
                    s = i - 2
                    z3p = ps3.tile([128, FD], F32, tag="z3")
                    for u in range(2):
                        nc.tensor.matmul(
                            z3p[u * 64 : u * 64 + 64, :],
                            w3t[:],
                            z2rs[s][:, u * FD : (u + 1) * FD],
                            start=True, stop=True,
                        )
                    del z2rs[s]
                    z3r = z3pool.tile([128, FD], BF16, tag="z3r")
                    relu_copy(z3r[:], z3p[:], b3v, on_act=True)    # ACT
                    z3rs[s] = z3r

                if i >= 3:
                    s = i - 3
                    # layer 4: supertile pair shares one PSUM tile
                    if s % 2 == 0:
                        z4p = ps4.tile([128, FD], F32, tag="z4")
                    half4 = (s % 2) * 64
                    nc.tensor.matmul(z4p[half4 : half4 + 64, :], w4t[:],
                                     z3rs[s][:], start=True, stop=True)
                    del z3rs[s]

                    if s % 2 == 1:
                        ze = zepool.tile([128, FD], F32, tag="ze")
                        nc.scalar.activation(ze[:], z4p[:], Exp, bias=b9v,
                                             scale=1.0)
                        zo = zopool.tile([128, FD], F32, tag="zo")
                        nc.scalar.activation(zo[:], ze[:], Ln, bias=1.0,
                                             scale=1.0)
                        nc.sync.dma_start(out[s // 2], zo[:])

    if split_waits:
        _split_multi_waits(nc)
    return nc


# ---------------------------------------------------------------- host side
def _block_weights(W1, W7, W8, W9):
    w1blk = np.zeros((96, 128), np.float32)
    for r in range(16):
        w1blk[r * 6 : r * 6 + 6, r * 8 : r * 8 + 8] = W1.T
    w2blk = np.zeros((128, 64), np.float32)
    for r in range(16):
        w2blk[r * 8 : r * 8 + 8, r * 4 : r * 4 + 4] = W7.T
    w3blk = np.zeros((128, 64), np.float32)
    for h in range(2):
        for r in range(16):
            w3blk[h * 64 + r * 4 : h * 64 + r * 4 + 4,
                  h * 32 + r * 2 : h * 32 + r * 2 + 2] = W8.T
    w4blk = np.zeros((128, 64), np.float32)
    for q in range(2):
        for h in range(2):
            for r in range(16):
                w4blk[q * 64 + h * 32 + r * 2 : q * 64 + h * 32 + r * 2 + 2,
                      q * 32 + h * 16 + r] = W9.T[:, 0]
    return w1blk, w2blk, w3blk, w4blk


def kernel(x, W1, b1, W7, b7, W8, b8, W9, b9):
    import ml_dtypes

    x = np.ascontiguousarray(np.asarray(x, dtype=np.float32))
    W1, b1 = np.asarray(W1, np.float32), np.asarray(b1, np.float32)
    W7, b7 = np.asarray(W7, np.float32), np.asarray(b7, np.float32)
    W8, b8 = np.asarray(W8, np.float32), np.asarray(b8, np.float32)
    W9, b9 = np.asarray(W9, np.float32), np.asarray(b9, np.float32)

    bf = ml_dtypes.bfloat16
    w1blk, w2blk, w3blk, w4blk = _block_weights(W1, W7, W8, W9)
    w1blk, w2blk = w1blk.astype(bf), w2blk.astype(bf)
    w3blk, w4blk = w3blk.astype(bf), w4blk.astype(bf)
    bvecs = np.stack(
        [
            b1[np.arange(128) % 8],
            b7[np.arange(128) % 4],
            b8[np.arange(128) % 2],
            np.full(128, float(b9[0])),
        ],
        axis=1,
    ).astype(np.float32)

    # [N,6] -> per core [n_st, 96, st_cols]: block layout, partition = r*6+k
    xb = (
        x.reshape(N_CORES, N_ST, ST_COLS, G, 6)
        .transpose(0, 1, 3, 4, 2)
        .reshape(N_CORES, N_ST, 96, ST_COLS)
        .astype(bf)
    )
    xb = np.ascontiguousarray(xb)

    nc = build_program(b9=float(b9[0]))
    in_maps = [
        {
            "xb": xb[c],
            "w1blk": w1blk,
            "w2blk": w2blk,
            "w3blk": w3blk,
            "w4blk": w4blk,
            "bvecs": bvecs,
        }
        for c in range(N_CORES)
    ]
    kwargs = {}
    if TRACE:
        _register_ntff_hook()
        kwargs["trace"] = True
    res = run_bass_kernel_spmd(nc, in_maps, list(range(N_CORES)), **kwargs)
    global LAST_RESULTS
    LAST_RESULTS = res

    # out[c] is [8, 128, 512]; row = ((((P*2+so)*2+q)*2+h)*512+c)*16+r
    outs = []
    for c in range(N_CORES):
        arr = res.results[c]["out"]
        arr = (
            arr.reshape(N_ST // 2, 2, 2, 2, G, FD)
            .transpose(0, 1, 2, 3, 5, 4)
            .reshape(ROWS_PER_CORE, 1)
        )
        outs.append(arr)
    return np.ascontiguousarray(np.concatenate(outs, axis=0))


# revision 17
# speedup vs baseline: 1.6387x; 1.0215x over previous
"""Trainium2 Bass kernel for nn_Encoder (dense MLP 6->8->4->2->1 + softplus).

Strategy: pure data parallel over 8 NeuronCores. The host pre-arranges x into
a "block" layout in which each group of 16 consecutive rows becomes one
streamed PE column (features x rows on SBUF partitions), so the whole MLP runs
as 4 chained block-diagonal matmuls on the TensorEngine with no on-chip
transposes. ReLU + bias are fused into the PSUM->SBUF evacuation copies
(split between VectorE and ScalarE); softplus = Ln(Exp(z)+1) on ScalarE.
Activations and x travel as bf16 (fp32 PSUM accumulation); output is fp32.
"""

import os
import sys

sys.path.insert(0, "/opt/trn_rl_repo")

import numpy as np

import concourse.bass as bass
import concourse.mybir as mybir
import concourse.tile as tile
from concourse.bass_utils import run_bass_kernel_spmd

# ---------------------------------------------------------------- geometry
N_CORES = 8
N_ROWS = 4194304
ROWS_PER_CORE = N_ROWS // N_CORES          # 524288
G = 16                                      # rows per PE column
COLS_PER_CORE = ROWS_PER_CORE // G          # 32768 columns of 16 rows
ST_COLS = 2048                              # columns per supertile
N_ST = COLS_PER_CORE // ST_COLS             # 16 supertiles per core
FD = 512                                    # matmul free dim / PSUM bank
BF16 = mybir.dt.bfloat16
F32 = mybir.dt.float32

# walrus in this container rejects instructions carrying more than
# _MAX_WAITS sync waits; split the surplus onto same-engine NoOps placed
# immediately before the instruction.
_MAX_WAITS = 1


def _split_multi_waits(nc, max_waits=_MAX_WAITS):
    ctr = 0
    for f in nc.m.functions:
        for bb in f.blocks:
            out = []
            for inst in bb.instructions:
                si = getattr(inst, "sync_info", None)
                if si is not None and si.on_wait and len(si.on_wait) > max_waits:
                    waits = list(si.on_wait)
                    split = len(waits) - max_waits
                    for i in range(0, split, max_waits):
                        nop = mybir.InstNoOp(
                            name=f"waitsplit-{ctr}", ins=[], outs=[]
                        )
                        ctr += 1
                        nop.engine = inst.engine
                        nop.sync_info = mybir.SyncInfo(
                            on_wait=waits[i : min(i + max_waits, split)],
                            on_update=[],
                        )
                        out.append(nop)
                    inst.sync_info = mybir.SyncInfo(
                        on_wait=waits[split:], on_update=list(si.on_update)
                    )
                out.append(inst)
            bb.instructions[:] = out


# Set KERNEL_TRACE=1 to neuron-profile the run; kernel() then stashes the
# BassKernelResults (exec_time_ns, trace paths) in LAST_RESULTS.
TRACE = os.environ.get("KERNEL_TRACE", "0") == "1"
LAST_RESULTS = None

# Let walrus dedupe back-to-back LDWEIGHTS of the same stationary (we reuse
# each weight matrix across consecutive matmuls); default-off flag upstream.
if os.environ.get("KLDW_OPT", "0") == "1":
    import concourse.bass_utils as _bu

    _orig_run_command = _bu.run_command

    def _run_command_ldwopt(cmd, *a, **kw):
        cmd = [
            c.replace("--enable-ldw-opt=false", "--enable-ldw-opt=true")
            if isinstance(c, str) else c
            for c in cmd
        ]
        return _orig_run_command(cmd, *a, **kw)

    _bu.run_command = _run_command_ldwopt


def _register_ntff_hook():
    """The image's antenv lacks axon_hooks; inject it and register the ctypes
    NTFF profile hook so run_bass_kernel_spmd(trace=True) works under axon."""
    import types

    if "antenv.axon_hooks" not in sys.modules:
        mod = types.ModuleType("antenv.axon_hooks")
        mod._hook = None

        def set_axon_ntff_profile_hook(h, _mod=mod):
            _mod._hook = h

        def get_axon_ntff_profile_hook(_mod=mod):
            return _mod._hook

        mod.set_axon_ntff_profile_hook = set_axon_ntff_profile_hook
        mod.get_axon_ntff_profile_hook = get_axon_ntff_profile_hook
        sys.modules["antenv.axon_hooks"] = mod
        import antenv

        antenv.axon_hooks = mod
    mod = sys.modules["antenv.axon_hooks"]
    if mod.get_axon_ntff_profile_hook() is None:
        try:
            from trn_agent_boot.trn_boot import _ntff_profile_via_ctypes

            mod.set_axon_ntff_profile_hook(
                _ntff_profile_via_ctypes("/opt/axon/libaxon_pjrt.so")
            )
        except Exception:
            pass


# ---------------------------------------------------------------- program
def build_program(n_st=N_ST, st_cols=ST_COLS, b9=0.0, split_waits=True,
                  use_softplus=False, warmup=16):
    """One SPMD NeuronCore program; all 8 cores run it on their own shard."""
    nc = bass.Bass("TRN2", target_bir_lowering=False, debug=False,
                   num_devices=N_CORES)

    xb = nc.dram_tensor("xb", [n_st, 96, st_cols], BF16,
                        kind="ExternalInput").ap()
    w1 = nc.dram_tensor("w1blk", [96, 128], BF16, kind="ExternalInput").ap()
    w2 = nc.dram_tensor("w2blk", [128, 64], BF16, kind="ExternalInput").ap()
    w3 = nc.dram_tensor("w3blk", [128, 64], BF16, kind="ExternalInput").ap()
    w4 = nc.dram_tensor("w4blk", [128, 64], BF16, kind="ExternalInput").ap()
    bv = nc.dram_tensor("bvecs", [128, 4], F32, kind="ExternalInput").ap()
    out = nc.dram_tensor("out", [n_st // 2, 128, FD], F32,
                         kind="ExternalOutput").ap()

    FD2 = 2 * FD
    Relu = mybir.ActivationFunctionType.Relu
    Exp = mybir.ActivationFunctionType.Exp
    Ln = mybir.ActivationFunctionType.Ln
    Softplus = mybir.ActivationFunctionType.Softplus
    ADD = mybir.AluOpType.add
    MAX = mybir.AluOpType.max

    with tile.TileContext(nc) as tc:
        with (
            tc.tile_pool(name="consts", bufs=1) as cpool,
            tc.tile_pool(name="xin", bufs=3) as xpool,
            tc.tile_pool(name="z1r", bufs=4) as z1pool,
            tc.tile_pool(name="z2r", bufs=3) as z2pool,
            tc.tile_pool(name="z3r", bufs=3) as z3pool,
            tc.tile_pool(name="zexp", bufs=2) as zepool,
            tc.tile_pool(name="zout", bufs=2) as zopool,
            tc.tile_pool(name="psbig", bufs=3, space="PSUM") as psbig,
            tc.tile_pool(name="ps3", bufs=1, space="PSUM") as ps3,
            tc.tile_pool(name="ps4", bufs=1, space="PSUM") as ps4,
        ):
            # --- constants into SBUF once
            w1t = cpool.tile([96, 128], BF16, tag="w1")
            nc.sync.dma_start(w1t[:], w1[:])
            w2t = cpool.tile([128, 64], BF16, tag="w2")
            nc.sync.dma_start(w2t[:], w2[:])
            w3t = cpool.tile([128, 64], BF16, tag="w3")
            nc.sync.dma_start(w3t[:], w3[:])
            w4t = cpool.tile([128, 64], BF16, tag="w4")
            nc.sync.dma_start(w4t[:], w4[:])
            bvt = cpool.tile([128, 4], F32, tag="bv")
            nc.sync.dma_start(bvt[:], bv[:])
            b1v, b2v, b3v, b9v = (bvt[:, 0:1], bvt[:, 1:2], bvt[:, 2:3],
                                  bvt[:, 3:4])

            # PE warmup: ~3.5us of dummy matmuls so the HAM clock-gate
            # reaches 8/8 while the first input DMAs are still in flight.
            if warmup:
                wscr = cpool.tile([96, FD], BF16, tag="wscr")
                nc.gpsimd.memset(wscr[:], 0.0)
                wps = ps4.tile([128, FD], F32, tag="z4")
                for _ in range(warmup):
                    nc.tensor.matmul(wps[:], wscr[:, 0:128], wscr[:],
                                     start=True, stop=True)

            def relu_copy(dst, src, bias_ap, on_act):
                if on_act:
                    nc.scalar.activation(dst, src, Relu, bias=bias_ap,
                                         scale=1.0)
                else:
                    nc.vector.tensor_scalar(dst, src, bias_ap, 0.0, ADD, MAX)

            # Software-pipelined across supertiles: in iteration i we emit
            # layer-1 for supertile i, layer-2 for i-1, layer-3 for i-2 and
            # layer-4 for i-3, so every matmul's inputs were produced a full
            # iteration earlier and the PE never stalls on a fresh copy.
            z1rs = {}
            z2rs = {}
            z3rs = {}
            z4p = None
            for i in range(n_st + 3):
                if i < n_st:
                    s = i
                    xt = xpool.tile([96, st_cols], BF16, tag="x")
                    nc.sync.dma_start(xt[:, :], xb[s])
                    # layer 1: two [96,128]^T @ [96,512] per 2-bank PSUM tile
                    z1rs[s] = []
                    for half in range(2):       # z1a (cols 0:1024), z1b
                        z1p = psbig.tile([128, FD2], F32, tag="zz")
                        for j in range(2):
                            c0 = half * FD2 + j * FD
                            nc.tensor.matmul(
                                z1p[:, j * FD : (j + 1) * FD],
                                w1t[:],
                                xt[:, c0 : c0 + FD],
                                start=True, stop=True,
                            )
                        z1r = z1pool.tile([128, FD2], BF16, tag="z1r")
                        relu_copy(z1r[:], z1p[:], b1v, on_act=(half == 1))
                        z1rs[s].append(z1r)

                if i >= 1 and i - 1 < n_st:
                    s = i - 1
                    # layer 2: 4 matmuls fill one 2-bank PSUM tile
                    z2p = psbig.tile([128, FD2], F32, tag="zz")
                    for half in range(2):
                        for j in range(2):
                            nc.tensor.matmul(
                                z2p[j * 64 : j * 64 + 64,
                                    half * FD : (half + 1) * FD],
                                w2t[:],
                                z1rs[s][half][:, j * FD : (j + 1) * FD],
                                start=True, stop=True,
                            )
                    del z1rs[s]
                    z2r = z2pool.tile([128, FD2], BF16, tag="z2r")
                    relu_copy(z2r[:], z2p[:], b2v, on_act=False)   # DVE
                    z2rs[s] = z2r

                if i >= 2 and i - 2 < n_st:
                    s = i - 2
                    z3p = ps3.tile([128, FD], F32, tag="z3")
                    for u in range(2):
                        nc.tensor.matmul(
                            z3p[u * 64 : u * 64 + 64, :],
                            w3t[:],
                            z2rs[s][:, u * FD : (u + 1) * FD],
                            start=True, stop=True,
                        )
                    del z2rs[s]
                    z3r = z3pool.tile([128, FD], BF16, tag="z3r")
                    relu_copy(z3r[:], z3p[:], b3v, on_act=True)    # ACT
                    z3rs[s] = z3r

                if i >= 3:
                    s = i - 3
                    # layer 4: supertile pair shares one PSUM tile
                    if s % 2 == 0:
                        z4p = ps4.tile([128, FD], F32, tag="z4")
                    half4 = (s % 2) * 64
                    nc.tensor.matmul(z4p[half4 : half4 + 64, :], w4t[:],
                                     z3rs[s][:], start=True, stop=True)
                    del z3rs[s]

                    if s % 2 == 1:
                        zo = zopool.tile([128, FD], F32, tag="zo")
                        if use_softplus:
                            nc.scalar.activation(zo[:], z4p[:], Softplus,
                                                 bias=b9v, scale=1.0)
                        else:
                            ze = zepool.tile([128, FD], F32, tag="ze")
                            nc.scalar.activation(ze[:], z4p[:], Exp, bias=b9v,
                                                 scale=1.0)
                            nc.scalar.activation(zo[:], ze[:], Ln, bias=1.0,
                                                 scale=1.0)
                        nc.sync.dma_start(out[s // 2], zo[:])

    if split_waits:
        _split_multi_waits(nc)
    return nc


# ---------------------------------------------------------------- host side
def _block_weights(W1, W7, W8, W9):
    w1blk = np.zeros((96, 128), np.float32)
    for r in range(16):
        w1blk[r * 6 : r * 6 + 6, r * 8 : r * 8 + 8] = W1.T
    w2blk = np.zeros((128, 64), np.float32)
    for r in range(16):
        w2blk[r * 8 : r * 8 + 8, r * 4 : r * 4 + 4] = W7.T
    w3blk = np.zeros((128, 64), np.float32)
    for h in range(2):
        for r in range(16):
            w3blk[h * 64 + r * 4 : h * 64 + r * 4 + 4,
                  h * 32 + r * 2 : h * 32 + r * 2 + 2] = W8.T
    w4blk = np.zeros((128, 64), np.float32)
    for q in range(2):
        for h in range(2):
            for r in range(16):
                w4blk[q * 64 + h * 32 + r * 2 : q * 64 + h * 32 + r * 2 + 2,
                      q * 32 + h * 16 + r] = W9.T[:, 0]
    return w1blk, w2blk, w3blk, w4blk


def kernel(x, W1, b1, W7, b7, W8, b8, W9, b9):
    import ml_dtypes

    x = np.ascontiguousarray(np.asarray(x, dtype=np.float32))
    W1, b1 = np.asarray(W1, np.float32), np.asarray(b1, np.float32)
    W7, b7 = np.asarray(W7, np.float32), np.asarray(b7, np.float32)
    W8, b8 = np.asarray(W8, np.float32), np.asarray(b8, np.float32)
    W9, b9 = np.asarray(W9, np.float32), np.asarray(b9, np.float32)

    bf = ml_dtypes.bfloat16
    w1blk, w2blk, w3blk, w4blk = _block_weights(W1, W7, W8, W9)
    w1blk, w2blk = w1blk.astype(bf), w2blk.astype(bf)
    w3blk, w4blk = w3blk.astype(bf), w4blk.astype(bf)
    bvecs = np.stack(
        [
            b1[np.arange(128) % 8],
            b7[np.arange(128) % 4],
            b8[np.arange(128) % 2],
            np.full(128, float(b9[0])),
        ],
        axis=1,
    ).astype(np.float32)

    # [N,6] -> per core [n_st, 96, st_cols]: block layout, partition = r*6+k
    xb = (
        x.reshape(N_CORES, N_ST, ST_COLS, G, 6)
        .transpose(0, 1, 3, 4, 2)
        .reshape(N_CORES, N_ST, 96, ST_COLS)
        .astype(bf)
    )
    xb = np.ascontiguousarray(xb)

    nc = build_program(b9=float(b9[0]))
    in_maps = [
        {
            "xb": xb[c],
            "w1blk": w1blk,
            "w2blk": w2blk,
            "w3blk": w3blk,
            "w4blk": w4blk,
            "bvecs": bvecs,
        }
        for c in range(N_CORES)
    ]
    kwargs = {}
    if TRACE:
        _register_ntff_hook()
        kwargs["trace"] = True
    res = run_bass_kernel_spmd(nc, in_maps, list(range(N_CORES)), **kwargs)
    global LAST_RESULTS
    LAST_RESULTS = res

    # out[c] is [8, 128, 512]; row = ((((P*2+so)*2+q)*2+h)*512+c)*16+r
    outs = []
    for c in range(N_CORES):
        arr = res.results[c]["out"]
        arr = (
            arr.reshape(N_ST // 2, 2, 2, 2, G, FD)
            .transpose(0, 1, 2, 3, 5, 4)
            .reshape(ROWS_PER_CORE, 1)
        )
        outs.append(arr)
    return np.ascontiguousarray(np.concatenate(outs, axis=0))
